# revision 32
# baseline (speedup 1.0000x reference)
"""Trainium2 Bass kernel for nn_Attention_28724741275862 — s5.

Factorization (see reference): per core (batch element)
    pp = W_enc @ enc^T (fp8, x256 scaled)   pd = W_dec @ dec^T (bf16)
    ee = exp(pp/256 + b)   ed = exp(pd)
    S = ee^T @ ed ; LSE = ln S
    ctx^T = pd*S_enc + (b*S_enc + C0') - enc^T @ LSE
      S_enc from DVE reduce of enc^T (x4 scaled fp8 -> /4)
      C0' = sum_j pp*enc^T / 1024

Schedule: A1 (fp8, early-landing chunks) runs before A2 (bf16 bt lands
~12us; its latency hides behind A1). ACT: ee exps right after A1
groups, ed after A2, Ln after C. DVE: S_enc/C0'/fix/ctmp all computed
before E completes; post-E work is only the 4 subtracts. Single ACT
table set (exp+ln). All DMAs dense partition-major.
"""

import sys

for _p in ("/opt/trn_rl_repo",):
    if _p not in sys.path:
        sys.path.insert(0, _p)

from contextlib import ExitStack

import numpy as np
import ml_dtypes

import concourse.bass as bass
import concourse.tile as tile
from concourse import bacc, mybir
from concourse.bass_utils import run_bass_kernel_spmd
from concourse.hw_specs import get_activation_tables

B, T_DEC, T_ENC, H2 = 8, 64, 512, 512
P = 128
NB = H2 // P

BF16 = mybir.dt.bfloat16
F32 = mybir.dt.float32
AF = mybir.ActivationFunctionType
ALU = mybir.AluOpType

BA_W = H2 + T_DEC

_CACHE = {}


def build_raw(bacc, mybir, bass):
    BF16 = mybir.dt.bfloat16
    F32 = mybir.dt.float32
    FP8 = mybir.dt.float8e4
    AF = mybir.ActivationFunctionType
    ALU = mybir.AluOpType

    nc = bacc.Bacc(None, target_bir_lowering=False)
    tabs = get_activation_tables(nc.m.arch)
    keep = "natural_log_exp_and_others"
    if keep in tabs and AF.Exp in tabs[keep] and AF.Ln in tabs[keep]:
        for name, st in tabs.items():
            if name != keep:
                st.discard(AF.Exp)
                st.discard(AF.Ln)

    encN = nc.dram_tensor("encN", [P, NB, H2], BF16, kind="ExternalInput")
    a1blk = nc.dram_tensor("a1blk", [NB, P, 2, H2], FP8, kind="ExternalInput")
    blkA = nc.dram_tensor("blkA", [P, NB, BA_W], BF16, kind="ExternalInput")
    b4d = nc.dram_tensor("b4", [P, NB], F32, kind="ExternalInput")
    out = nc.dram_tensor("out", [H2, T_DEC], F32, kind="ExternalOutput")
    out_r = out[:, :].rearrange("(a p) i -> p a i", p=P)

    with ExitStack() as ctx:
        ec = ctx.enter_context
        a1t = [ec(nc.sbuf_tensor(f"a1t{d}", [P, 2, H2], FP8)) for d in range(NB)]
        bt = ec(nc.sbuf_tensor("bt", [P, NB, BA_W], BF16))
        eN = ec(nc.sbuf_tensor("eN", [P, NB, H2], BF16))
        b4 = ec(nc.sbuf_tensor("b4s", [P, NB], F32))
        ee = [ec(nc.sbuf_tensor(f"ee{o}", [P, T_ENC], BF16)) for o in range(NB)]
        ed = [ec(nc.sbuf_tensor(f"ed{o}", [P, T_DEC], BF16)) for o in range(NB)]
        lt = [ec(nc.sbuf_tensor(f"lt{j}", [P, T_DEC + 1], BF16)) for j in range(NB)]
        junk = ec(nc.sbuf_tensor("junk", [P, NB, T_ENC], F32))
        jbf = ec(nc.sbuf_tensor("jbf", [P, T_ENC], BF16))
        wj = ec(nc.sbuf_tensor("wj", [P, NB], F32))
        red = ec(nc.sbuf_tensor("red", [P, NB], F32))
        se = ec(nc.sbuf_tensor("se", [P, NB], F32))
        cp = ec(nc.sbuf_tensor("cp", [P, NB], F32))
        cps = ec(nc.sbuf_tensor("cps", [P, NB], F32))
        fx = ec(nc.sbuf_tensor("fx", [P, NB], F32))
        ctmp = ec(nc.sbuf_tensor("ctmp", [P, NB, T_DEC], F32))
        ctxo = ec(nc.sbuf_tensor("ctxo", [P, NB, T_DEC], F32))
        pp = [ec(nc.psum_tensor(f"pp{o}", [P, T_ENC], F32)) for o in range(NB)]
        pd = ec(nc.psum_tensor("pd", [P, NB, T_DEC], F32))
        ps = ec(nc.psum_tensor("ps", [P, NB, T_DEC], F32))
        pcA = ec(nc.psum_tensor("pcA", [P, 2, T_DEC + 1], F32))
        pcB = ec(nc.psum_tensor("pcB", [P, 2, T_DEC + 1], F32))

        def pc(ob):
            return (pcA if ob < 2 else pcB)[:, ob % 2, :]

        def wte(db):
            return a1t[db][:, 0, :]

        def eT(db):
            return a1t[db][:, 1, :]

        jz = ec(nc.semaphore("jz"))
        dS = [ec(nc.semaphore(f"dS{d}")) for d in range(NB)]
        gA1 = ec(nc.semaphore("gA1"))
        gA2 = ec(nc.semaphore("gA2"))
        gA3 = ec(nc.semaphore("gA3"))
        dO = ec(nc.semaphore("dO"))
        pe = ec(nc.semaphore("pe"))
        ac = ec(nc.semaphore("ac"))
        dv = ec(nc.semaphore("dv"))

        # pe: A1 per-ob 1..4, A2 per-ob 5..8, C per-jb 9..12, E per-ob 13..16
        # ac: warmup 1, ee 2..5, ed 6..9, ln 10..13
        # dv: jbf 1, ones 2..5, se 6..13, C0' 14..25, fix/ctmp 26..33,
        #     subs 34..37
        dvc = {"n": 0}
        dvthr_ones = 4
        dvthr_out01 = 26
        dvthr_out23 = 28

        with nc.Block(no_gpsimd_drain=True) as block:

            @block.sync
            def _(sync):
                for db in range(NB):
                    sync.dma_start(
                        out=a1t[db][:, :, :], in_=a1blk[db, :, :, :]
                    ).then_inc(dS[db], 16)
                sync.wait_ge(dv, dvthr_out23)
                sync.dma_start(out=out_r[:, :, :], in_=ctxo[:, :, :]).then_inc(
                    dO, 16
                )
                sync.wait_ge(dO, 16)

            @block.gpsimd
            def _(gpsimd):
                gpsimd.wait_ge(dS[1], 16)
                gpsimd.dma_start(out=eN[:, :, :], in_=encN[:, :, :]).then_inc(
                    gA3, 16
                )

            @block.scalar
            def _(scalar):
                scalar.activation(wj[:, 0:1], wj[:, 3:4], AF.Exp, scale=0.0).then_inc(
                    ac, 1
                )  # ac=1
                scalar.dma_start(out=bt[:, :, :], in_=blkA[:, :, :]).then_inc(gA1, 16)
                scalar.dma_start(out=b4[:, :], in_=b4d[:, :]).then_inc(gA2, 16)
                # ee = exp(pp/256 + b) as A1 per-ob groups complete
                scalar.wait_ge(gA2, 16)
                for ob in range(NB):
                    scalar.wait_ge(pe, 1 + ob)
                    scalar.activation(
                        ee[ob][:, :],
                        pp[ob][:, :],
                        AF.Exp,
                        scale=1.0 / 256,
                        bias=b4[:, ob : ob + 1],
                    ).then_inc(ac, 1)  # ac 2..5
                # ed = exp(pd) as A2 per-ob groups complete
                for ob in range(NB):
                    scalar.wait_ge(pe, 5 + ob)
                    scalar.activation(ed[ob][:, :], pd[:, ob, :], AF.Exp).then_inc(
                        ac, 1
                    )  # ac 6..9
                # LSE = ln(S) after ALL C (ACT must not read the ps bank
                # while PE still writes other jb slices of it)
                scalar.wait_ge(pe, 12)
                for jb in range(NB):
                    scalar.activation(lt[jb][:, 0:T_DEC], ps[:, jb, :], AF.Ln).then_inc(
                        ac, 1
                    )  # ac 10..13

            @block.tensor
            def _(tensor):
                tensor.wait_ge(jz, 1)
                # continuous junk burst flips HAM to 8/8 while a1t chunks land
                for k in range(7):
                    tensor.matmul(
                        pp[k % NB][:, :],
                        lhsT=jbf[:, 0:P],
                        rhs=jbf[:, :],
                        start=True,
                        stop=True,
                    )
                # A1 first: fp8 chunks land early
                for db in range(NB):
                    tensor.wait_ge(dS[db], 16)
                    for ob in range(NB):
                        mm = tensor.matmul(
                            pp[ob][:, :],
                            lhsT=wte(db)[:, ob * P : (ob + 1) * P],
                            rhs=eT(db)[:, :],
                            start=(db == 0),
                            stop=(db == NB - 1),
                        )
                        if db == NB - 1:
                            mm.then_inc(pe, 1)  # pe 1..4
                # A2 behind A1 (bt latency hidden)
                tensor.wait_ge(gA1, 16)
                for ob in range(NB):
                    for db in range(NB):
                        mm = tensor.matmul(
                            pd[:, ob, :],
                            lhsT=bt[:, db, ob * P : (ob + 1) * P],
                            rhs=bt[:, db, H2 : H2 + T_DEC],
                            start=(db == 0),
                            stop=(db == NB - 1),
                        )
                        if db == NB - 1:
                            mm.then_inc(pe, 1)  # pe 5..8
                # C: S += ee^T @ ed
                tensor.wait_ge(ac, 9)
                for jb in range(NB):
                    for ob in range(NB):
                        mm = tensor.matmul(
                            ps[:, jb, :],
                            lhsT=ee[ob][:, jb * P : (jb + 1) * P],
                            rhs=ed[ob][:, :],
                            start=(ob == 0),
                            stop=(ob == NB - 1),
                        )
                        if ob == NB - 1:
                            mm.then_inc(pe, 1)  # pe 9..12
                # E: [ctx2 | S_enc] += enc^T.T @ [LSE | 1]
                tensor.wait_ge(gA3, 16)
                tensor.wait_ge(dv, dvthr_ones)
                for ob in range(NB):
                    for jb in range(NB):
                        if ob == 0:
                            tensor.wait_ge(ac, 10 + jb)
                        mm = tensor.matmul(
                            pc(ob),
                            lhsT=eN[:, jb, ob * P : (ob + 1) * P],
                            rhs=lt[jb][:, :],
                            start=(jb == 0),
                            stop=(jb == NB - 1),
                        )
                        if jb == NB - 1:
                            mm.then_inc(pe, 1)  # pe 13..16

            @block.vector
            def _(vector):
                def inc(instr):
                    instr.then_inc(dv, 1)
                    dvc["n"] += 1
                    return dvc["n"]

                vector.memset(jbf[:, :], 0.0).then_inc(jz, 1)
                for jb in range(NB):
                    inc(vector.memset(lt[jb][:, T_DEC : T_DEC + 1], 1.0))
                # S_enc = reduce(enc^T * 4) / 4, per chunk as it lands
                for ob in range(NB):
                    vector.wait_ge(dS[ob], 16)
                    r = inc(
                        vector.reduce_sum(
                            out=red[:, ob : ob + 1],
                            in_=eT(ob)[:, :],
                            axis=mybir.AxisListType.X,
                        )
                    )
                    vector.wait_ge(dv, r)
                    inc(
                        vector.tensor_scalar(
                            out=se[:, ob : ob + 1],
                            in0=red[:, ob : ob + 1],
                            scalar1=0.25,
                            scalar2=None,
                            op0=ALU.mult,
                        )
                    )
                # C0' fused: junk = (pp*1/1024)*eT ; cp = sum_j (true C0')
                c0dv = []
                for ob in range(NB):
                    vector.wait_ge(ac, 2 + ob)
                    c0dv.append(inc(
                        vector.scalar_tensor_tensor(
                            out=junk[:, ob, :],
                            in0=pp[ob][:, :],
                            scalar=1.0 / 1024,
                            in1=eT(ob)[:, :],
                            op0=ALU.mult,
                            op1=ALU.mult,
                            accum_out=cp[:, ob : ob + 1],
                        )
                    ))
                # fix = b*S_enc + C0'; ctmp = pd*S_enc + fix  (all pre-E)
                vector.wait_ge(gA2, 16)
                for ob in range(NB):
                    vector.wait_ge(dv, c0dv[ob])
                    f = inc(
                        vector.tensor_scalar(
                            out=fx[:, ob : ob + 1],
                            in0=se[:, ob : ob + 1],
                            scalar1=b4[:, ob : ob + 1],
                            scalar2=cp[:, ob : ob + 1],
                            op0=ALU.mult,
                            op1=ALU.add,
                        )
                    )
                    vector.wait_ge(pe, 5 + ob)  # pd[ob] done (A2)
                    vector.wait_ge(dv, f)
                    inc(
                        vector.tensor_scalar(
                            out=ctmp[:, ob, :],
                            in0=pd[:, ob, :],
                            scalar1=se[:, ob : ob + 1],
                            scalar2=fx[:, ob : ob + 1],
                            op0=ALU.mult,
                            op1=ALU.add,
                        )
                    )
                # post-E: only the subtracts
                tprev = dvc["n"]
                vector.wait_ge(pe, 14)
                for ob in (0, 1):
                    vector.wait_ge(dv, tprev)
                    inc(
                        vector.tensor_tensor(
                            out=ctxo[:, ob, :],
                            in0=ctmp[:, ob, :],
                            in1=pc(ob)[:, 0:T_DEC],
                            op=ALU.subtract,
                        )
                    )
                vector.wait_ge(pe, 16)
                for ob in (2, 3):
                    inc(
                        vector.tensor_tensor(
                            out=ctxo[:, ob, :],
                            in0=ctmp[:, ob, :],
                            in1=pc(ob)[:, 0:T_DEC],
                            op=ALU.subtract,
                        )
                    )

        nc.finalize()
    return nc


def _build_nc():
    return build_raw(bacc, mybir, bass)


def _prep_in_maps(encoderOutput, decoderInput, W, b):
    bf = ml_dtypes.bfloat16
    f8 = ml_dtypes.float8_e4m3fn
    WT = np.ascontiguousarray(np.asarray(W, np.float32).T)
    b4 = np.ascontiguousarray(np.asarray(b, np.float32).reshape(NB, P).T)
    in_maps = []
    for core in range(B):
        e = np.asarray(encoderOutput[core], np.float32)
        d = np.asarray(decoderInput[core], np.float32)
        eT = e.T
        dT = d.T
        a1 = np.empty((NB, P, 2, H2), np.float32)
        a1[:, :, 0] = 64.0 * WT[H2:].reshape(NB, P, H2)
        a1[:, :, 1] = 4.0 * eT.reshape(NB, P, T_ENC)
        bA = np.empty((P, NB, H2 + T_DEC), np.float32)
        bA[:, :, :H2] = WT[:H2].reshape(NB, P, H2).transpose(1, 0, 2)
        bA[:, :, H2:] = dT.reshape(NB, P, T_DEC).transpose(1, 0, 2)
        enj = np.ascontiguousarray(e.reshape(NB, P, H2).transpose(1, 0, 2))
        in_maps.append(
            {
                "encN": enj.astype(bf),
                "a1blk": a1.astype(f8),
                "blkA": bA.astype(bf),
                "b4": b4,
            }
        )
    return in_maps


def kernel(encoderOutput, decoderInput, W, b, _trace=False):
    if "nc" not in _CACHE:
        _CACHE["nc"] = _build_nc()
    nc = _CACHE["nc"]
    in_maps = _prep_in_maps(encoderOutput, decoderInput, W, b)
    res = run_bass_kernel_spmd(nc, in_maps, core_ids=list(range(B)), trace=_trace)
    outs = np.stack([np.asarray(r["out"], np.float32).T for r in res.results])
    if _trace:
        _CACHE["last_result"] = res
    return outs


# revision 33
# speedup vs baseline: 1.1841x; 1.1841x over previous
"""Trainium2 Bass kernel for nn_Attention_28724741275862 — s5.

Factorization (see reference): per core (batch element)
    pp = W_enc @ enc^T (fp8, x256 scaled)   pd = W_dec @ dec^T (bf16)
    ee = exp(pp/256 + b)   ed = exp(pd)
    S = ee^T @ ed ; LSE = ln S
    ctx^T = pd*S_enc + (b*S_enc + C0') - enc^T @ LSE
      S_enc from DVE reduce of enc^T (x4 scaled fp8 -> /4)
      C0' = sum_j pp*enc^T / 1024

Schedule: A1 (fp8, early-landing chunks) runs before A2 (bf16 bt lands
~12us; its latency hides behind A1). ACT: ee exps right after A1
groups, ed after A2, Ln after C. DVE: S_enc/C0'/fix/ctmp all computed
before E completes; post-E work is only the 4 subtracts. Single ACT
table set (exp+ln). All DMAs dense partition-major.
"""

import sys

for _p in ("/opt/trn_rl_repo",):
    if _p not in sys.path:
        sys.path.insert(0, _p)

from contextlib import ExitStack

import numpy as np
import ml_dtypes

import concourse.bass as bass
import concourse.tile as tile
from concourse import bacc, mybir
from concourse.bass_utils import run_bass_kernel_spmd
from concourse.hw_specs import get_activation_tables

B, T_DEC, T_ENC, H2 = 8, 64, 512, 512
P = 128
NB = H2 // P

BF16 = mybir.dt.bfloat16
F32 = mybir.dt.float32
AF = mybir.ActivationFunctionType
ALU = mybir.AluOpType

BA_W = H2 + T_DEC

_CACHE = {}


def build_raw(bacc, mybir, bass):
    BF16 = mybir.dt.bfloat16
    F32 = mybir.dt.float32
    FP8 = mybir.dt.float8e4
    AF = mybir.ActivationFunctionType
    ALU = mybir.AluOpType

    nc = bacc.Bacc(None, target_bir_lowering=False)
    tabs = get_activation_tables(nc.m.arch)
    keep = "natural_log_exp_and_others"
    if keep in tabs and AF.Exp in tabs[keep] and AF.Ln in tabs[keep]:
        for name, st in tabs.items():
            if name != keep:
                st.discard(AF.Exp)
                st.discard(AF.Ln)

    encN = nc.dram_tensor("encN", [P, NB, H2], BF16, kind="ExternalInput")
    a1blk = nc.dram_tensor("a1blk", [NB, P, 2, H2], FP8, kind="ExternalInput")
    blkA = nc.dram_tensor("blkA", [P, NB, BA_W], BF16, kind="ExternalInput")
    b4d = nc.dram_tensor("b4", [P, NB], F32, kind="ExternalInput")
    out = nc.dram_tensor("out", [H2, T_DEC], F32, kind="ExternalOutput")
    out_r = out[:, :].rearrange("(a p) i -> p a i", p=P)

    with ExitStack() as ctx:
        ec = ctx.enter_context
        a1t = [ec(nc.sbuf_tensor(f"a1t{d}", [P, 2, H2], FP8)) for d in range(NB)]
        bt = ec(nc.sbuf_tensor("bt", [P, NB, BA_W], BF16))
        eN = ec(nc.sbuf_tensor("eN", [P, NB, H2], BF16))
        b4 = ec(nc.sbuf_tensor("b4s", [P, NB], F32))
        ee = [ec(nc.sbuf_tensor(f"ee{o}", [P, T_ENC], BF16)) for o in range(NB)]
        ed = [ec(nc.sbuf_tensor(f"ed{o}", [P, T_DEC], BF16)) for o in range(NB)]
        lt = [ec(nc.sbuf_tensor(f"lt{j}", [P, T_DEC + 1], BF16)) for j in range(NB)]
        junk = ec(nc.sbuf_tensor("junk", [P, NB, T_ENC], F32))
        jbf = ec(nc.sbuf_tensor("jbf", [P, T_ENC], BF16))
        wj = ec(nc.sbuf_tensor("wj", [P, NB], F32))
        red = ec(nc.sbuf_tensor("red", [P, NB], F32))
        se = ec(nc.sbuf_tensor("se", [P, NB], F32))
        cp = ec(nc.sbuf_tensor("cp", [P, NB], F32))
        cps = ec(nc.sbuf_tensor("cps", [P, NB], F32))
        fx = ec(nc.sbuf_tensor("fx", [P, NB], F32))
        ctmp = ec(nc.sbuf_tensor("ctmp", [P, NB, T_DEC], F32))
        ctxo = ec(nc.sbuf_tensor("ctxo", [P, NB, T_DEC], F32))
        pp = [ec(nc.psum_tensor(f"pp{o}", [P, T_ENC], F32)) for o in range(NB)]
        pd = ec(nc.psum_tensor("pd", [P, NB, T_DEC], F32))
        ps = ec(nc.psum_tensor("ps", [P, NB, T_DEC], F32))
        pcA = ec(nc.psum_tensor("pcA", [P, 2, T_DEC + 1], F32))
        pcB = ec(nc.psum_tensor("pcB", [P, 2, T_DEC + 1], F32))

        def pc(ob):
            return (pcA if ob < 2 else pcB)[:, ob % 2, :]

        def wte(db):
            return a1t[db][:, 0, :]

        def eT(db):
            return a1t[db][:, 1, :]

        dS = [ec(nc.semaphore(f"dS{d}")) for d in range(NB)]
        gA1 = ec(nc.semaphore("gA1"))
        gA2 = ec(nc.semaphore("gA2"))
        gA3 = ec(nc.semaphore("gA3"))
        dO = ec(nc.semaphore("dO"))
        pe = ec(nc.semaphore("pe"))
        ac = ec(nc.semaphore("ac"))
        dv = ec(nc.semaphore("dv"))

        # pe: A1 per-ob 1..4, A2 per-ob 5..8, C per-jb 9..12, E per-ob 13..16
        # ac: warmup 1, ee 2..5, ed 6..9, ln 10..13
        # dv: jbf 1, ones 2..5, se 6..13, C0' 14..25, fix/ctmp 26..33,
        #     subs 34..37
        dvc = {"n": 0}
        dvthr_ones = 5
        dvthr_out01 = 27
        dvthr_out23 = 29

        with nc.Block(no_gpsimd_drain=True) as block:

            @block.sync
            def _(sync):
                for db in range(NB):
                    sync.dma_start(
                        out=a1t[db][:, :, :], in_=a1blk[db, :, :, :]
                    ).then_inc(dS[db], 16)
                sync.wait_ge(dv, dvthr_out01)
                sync.dma_start(out=out_r[:, 0:2, :], in_=ctxo[:, 0:2, :]).then_inc(
                    dO, 16
                )
                sync.wait_ge(dv, dvthr_out23)
                sync.dma_start(out=out_r[:, 2:NB, :], in_=ctxo[:, 2:NB, :]).then_inc(
                    dO, 16
                )
                sync.wait_ge(dO, 32)

            @block.gpsimd
            def _(gpsimd):
                gpsimd.wait_ge(dS[1], 16)
                gpsimd.dma_start(out=eN[:, :, :], in_=encN[:, :, :]).then_inc(
                    gA3, 16
                )

            @block.scalar
            def _(scalar):
                scalar.activation(wj[:, 0:1], wj[:, 3:4], AF.Exp, scale=0.0).then_inc(
                    ac, 1
                )  # ac=1
                scalar.dma_start(out=bt[:, :, :], in_=blkA[:, :, :]).then_inc(gA1, 16)
                scalar.dma_start(out=b4[:, :], in_=b4d[:, :]).then_inc(gA2, 16)
                # ee = exp(pp/256 + b) as A1 per-ob groups complete
                scalar.wait_ge(gA2, 16)
                for ob in range(NB):
                    scalar.wait_ge(pe, 1 + ob)
                    scalar.activation(
                        ee[ob][:, :],
                        pp[ob][:, :],
                        AF.Exp,
                        scale=1.0 / 256,
                        bias=b4[:, ob : ob + 1],
                    ).then_inc(ac, 1)  # ac 2..5
                # ed = exp(pd) as A2 per-ob groups complete
                for ob in range(NB):
                    scalar.wait_ge(pe, 5 + ob)
                    scalar.activation(ed[ob][:, :], pd[:, ob, :], AF.Exp).then_inc(
                        ac, 1
                    )  # ac 6..9
                # LSE = ln(S) after all C
                scalar.wait_ge(pe, 12)
                for jb in range(NB):
                    scalar.activation(lt[jb][:, 0:T_DEC], ps[:, jb, :], AF.Ln).then_inc(
                        ac, 1
                    )  # ac 10..13

            @block.tensor
            def _(tensor):
                tensor.wait_ge(dv, 1)
                # one continuous ~3.9us junk burst flips HAM to 8/8 while
                # the a1t chunks land
                for k in range(9):
                    tensor.matmul(
                        pp[k % NB][:, :],
                        lhsT=jbf[:, 0:P],
                        rhs=jbf[:, :],
                        start=True,
                        stop=True,
                    )
                # A1 first: fp8 chunks land early
                for db in range(NB):
                    tensor.wait_ge(dS[db], 16)
                    for ob in range(NB):
                        mm = tensor.matmul(
                            pp[ob][:, :],
                            lhsT=wte(db)[:, ob * P : (ob + 1) * P],
                            rhs=eT(db)[:, :],
                            start=(db == 0),
                            stop=(db == NB - 1),
                        )
                        if db == NB - 1:
                            mm.then_inc(pe, 1)  # pe 1..4
                # A2 behind A1 (bt latency hidden)
                tensor.wait_ge(gA1, 16)
                for ob in range(NB):
                    for db in range(NB):
                        mm = tensor.matmul(
                            pd[:, ob, :],
                            lhsT=bt[:, db, ob * P : (ob + 1) * P],
                            rhs=bt[:, db, H2 : H2 + T_DEC],
                            start=(db == 0),
                            stop=(db == NB - 1),
                        )
                        if db == NB - 1:
                            mm.then_inc(pe, 1)  # pe 5..8
                # C: S += ee^T @ ed
                tensor.wait_ge(ac, 9)
                for jb in range(NB):
                    for ob in range(NB):
                        mm = tensor.matmul(
                            ps[:, jb, :],
                            lhsT=ee[ob][:, jb * P : (jb + 1) * P],
                            rhs=ed[ob][:, :],
                            start=(ob == 0),
                            stop=(ob == NB - 1),
                        )
                        if ob == NB - 1:
                            mm.then_inc(pe, 1)  # pe 9..12
                # E: [ctx2 | S_enc] += enc^T.T @ [LSE | 1]
                tensor.wait_ge(gA3, 16)
                tensor.wait_ge(dv, dvthr_ones)
                for ob in range(NB):
                    for jb in range(NB):
                        if ob == 0:
                            tensor.wait_ge(ac, 10 + jb)
                        mm = tensor.matmul(
                            pc(ob),
                            lhsT=eN[:, jb, ob * P : (ob + 1) * P],
                            rhs=lt[jb][:, :],
                            start=(jb == 0),
                            stop=(jb == NB - 1),
                        )
                        if jb == NB - 1:
                            mm.then_inc(pe, 1)  # pe 13..16

            @block.vector
            def _(vector):
                def inc(instr):
                    instr.then_inc(dv, 1)
                    dvc["n"] += 1
                    return dvc["n"]

                inc(vector.memset(jbf[:, :], 0.0))  # dv=1
                for jb in range(NB):
                    inc(vector.memset(lt[jb][:, T_DEC : T_DEC + 1], 1.0))
                # S_enc = reduce(enc^T * 4) / 4, per chunk as it lands
                for ob in range(NB):
                    vector.wait_ge(dS[ob], 16)
                    r = inc(
                        vector.reduce_sum(
                            out=red[:, ob : ob + 1],
                            in_=eT(ob)[:, :],
                            axis=mybir.AxisListType.X,
                        )
                    )
                    vector.wait_ge(dv, r)
                    inc(
                        vector.tensor_scalar(
                            out=se[:, ob : ob + 1],
                            in0=red[:, ob : ob + 1],
                            scalar1=0.25,
                            scalar2=None,
                            op0=ALU.mult,
                        )
                    )
                # C0' fused: junk = (pp*1/1024)*eT ; cp = sum_j (true C0')
                c0dv = []
                for ob in range(NB):
                    vector.wait_ge(ac, 2 + ob)
                    c0dv.append(inc(
                        vector.scalar_tensor_tensor(
                            out=junk[:, ob, :],
                            in0=pp[ob][:, :],
                            scalar=1.0 / 1024,
                            in1=eT(ob)[:, :],
                            op0=ALU.mult,
                            op1=ALU.mult,
                            accum_out=cp[:, ob : ob + 1],
                        )
                    ))
                # fix = b*S_enc + C0'; ctmp = pd*S_enc + fix  (all pre-E)
                vector.wait_ge(gA2, 16)
                for ob in range(NB):
                    vector.wait_ge(dv, c0dv[ob])
                    f = inc(
                        vector.tensor_scalar(
                            out=fx[:, ob : ob + 1],
                            in0=se[:, ob : ob + 1],
                            scalar1=b4[:, ob : ob + 1],
                            scalar2=cp[:, ob : ob + 1],
                            op0=ALU.mult,
                            op1=ALU.add,
                        )
                    )
                    vector.wait_ge(pe, 5 + ob)  # pd[ob] done (A2)
                    vector.wait_ge(dv, f)
                    inc(
                        vector.tensor_scalar(
                            out=ctmp[:, ob, :],
                            in0=pd[:, ob, :],
                            scalar1=se[:, ob : ob + 1],
                            scalar2=fx[:, ob : ob + 1],
                            op0=ALU.mult,
                            op1=ALU.add,
                        )
                    )
                # post-E: only the subtracts
                tprev = dvc["n"]
                vector.wait_ge(pe, 14)
                for ob in (0, 1):
                    vector.wait_ge(dv, tprev)
                    inc(
                        vector.tensor_tensor(
                            out=ctxo[:, ob, :],
                            in0=ctmp[:, ob, :],
                            in1=pc(ob)[:, 0:T_DEC],
                            op=ALU.subtract,
                        )
                    )
                vector.wait_ge(pe, 16)
                for ob in (2, 3):
                    inc(
                        vector.tensor_tensor(
                            out=ctxo[:, ob, :],
                            in0=ctmp[:, ob, :],
                            in1=pc(ob)[:, 0:T_DEC],
                            op=ALU.subtract,
                        )
                    )

        nc.finalize()
    return nc


def _build_nc():
    return build_raw(bacc, mybir, bass)


def _prep_in_maps(encoderOutput, decoderInput, W, b):
    bf = ml_dtypes.bfloat16
    f8 = ml_dtypes.float8_e4m3fn
    WT = np.ascontiguousarray(np.asarray(W, np.float32).T)
    b4 = np.ascontiguousarray(np.asarray(b, np.float32).reshape(NB, P).T)
    in_maps = []
    for core in range(B):
        e = np.asarray(encoderOutput[core], np.float32)
        d = np.asarray(decoderInput[core], np.float32)
        eT = e.T
        dT = d.T
        a1 = np.empty((NB, P, 2, H2), np.float32)
        a1[:, :, 0] = 64.0 * WT[H2:].reshape(NB, P, H2)
        a1[:, :, 1] = 4.0 * eT.reshape(NB, P, T_ENC)
        bA = np.empty((P, NB, H2 + T_DEC), np.float32)
        bA[:, :, :H2] = WT[:H2].reshape(NB, P, H2).transpose(1, 0, 2)
        bA[:, :, H2:] = dT.reshape(NB, P, T_DEC).transpose(1, 0, 2)
        enj = np.ascontiguousarray(e.reshape(NB, P, H2).transpose(1, 0, 2))
        in_maps.append(
            {
                "encN": enj.astype(bf),
                "a1blk": a1.astype(f8),
                "blkA": bA.astype(bf),
                "b4": b4,
            }
        )
    return in_maps


def kernel(encoderOutput, decoderInput, W, b, _trace=False):
    if "nc" not in _CACHE:
        _CACHE["nc"] = _build_nc()
    nc = _CACHE["nc"]
    in_maps = _prep_in_maps(encoderOutput, decoderInput, W, b)
    res = run_bass_kernel_spmd(nc, in_maps, core_ids=list(range(B)), trace=_trace)
    outs = np.stack([np.asarray(r["out"], np.float32).T for r in res.results])
    if _trace:
        _CACHE["last_result"] = res
    return outs
